# revision 3
# baseline (speedup 1.0000x reference)
"""Trainium2 Bass kernel for the KSubspaceBaseModel objective.

Reference computes, for B=2048 samples x (B, D=1024) and subspace bases
Us (R=4, K=16, D, d=32):
    z = x @ U; x_ = z @ U^T; loss = 0.5*||x - x_||^2  (per b, r, k)
    obj_r = mean_b min_k loss

Algebraic collapse used here: with G = U^T U,
    loss = 0.5||x||^2 - z^T (I - 0.5 G) z
Folding L = chol(I - 0.5G) into U (Ut = U @ L) host-side gives
    loss = 0.5||x||^2 - ||Ut^T x||^2
so the device only computes z~ = Ut^T x, squares it, sums each subspace's
32 latent columns, and takes max_k.  obj_r = 0.5*mean||x||^2 - mean_b max_k.

Sharding over 8 cores: 4 batch quarters (512 samples) x 2 subspace halves
(32 subspaces = 2 whole replicates), so the k-max is core-local.
Device layout: stationary = x^T chunks (contraction D on partitions),
moving = Ut.  z~ lands [batch(128) x latent] in PSUM, so per-subspace sums
and the k-max are free-dim reductions.
"""

import numpy as np
import ml_dtypes

import concourse.bass as bass
import concourse.bacc as bacc
import concourse.mybir as mybir
import concourse.tile as tile
from concourse.bass_utils import run_bass_kernel_spmd

B, D, R, K, d = 2048, 1024, 4, 16, 32
NCORES = 8
NB = B // 4          # 512 samples per core
NS = 32              # subspaces per core (2 replicates)
KC = D // 128        # 8 contraction chunks
BC = NB // 128       # 4 batch chunks per core
BF16 = mybir.dt.bfloat16
FP32 = mybir.dt.float32

_COMPILED = {}
LAST_RESULTS = None


def _build():
    nc = bacc.Bacc("TRN2", target_bir_lowering=False, debug=False)
    xt = nc.dram_tensor("xt", [D, NB], BF16, kind="ExternalInput")
    ut = nc.dram_tensor("ut", [D, NS * d], BF16, kind="ExternalInput")
    mx = nc.dram_tensor("mx", [2, NB], FP32, kind="ExternalOutput")
    xsq = nc.dram_tensor("xsq", [128, KC], FP32, kind="ExternalOutput")

    xt_v = xt.ap().rearrange("(o p) n -> p o n", p=128)   # [128, KC, NB]
    ut_v = ut.ap().rearrange("(o p) n -> p o n", p=128)   # [128, KC, 1024]

    with tile.TileContext(nc) as tc:
        with (
            tc.tile_pool(name="xsb", bufs=1) as xpool,
            tc.tile_pool(name="usb", bufs=1) as upool,
            tc.tile_pool(name="esb", bufs=3) as epool,
            tc.tile_pool(name="asb", bufs=1) as apool,
            tc.tile_pool(name="msb", bufs=1) as mpool,
            tc.tile_pool(name="sqsb", bufs=2) as sqpool,
            tc.tile_pool(name="single", bufs=1) as spool,
            tc.tile_pool(name="zp", bufs=2, space="PSUM") as zpool,
        ):
            # ut halves first (they gate the first matmul group), xt interleaved
            u_sb = [[None] * KC for _ in range(2)]
            x_sb = [None] * KC
            for kc in range(KC):
                u_sb[0][kc] = upool.tile([128, 512], BF16, tag=f"u0_{kc}", name=f"u0_{kc}")
                nc.sync.dma_start(u_sb[0][kc][:], ut_v[:, kc, 0:512])
                x_sb[kc] = xpool.tile([128, NB], BF16, tag=f"x_{kc}", name=f"x_{kc}")
                nc.sync.dma_start(x_sb[kc][:], xt_v[:, kc, :])
            for kc in range(KC):
                u_sb[1][kc] = upool.tile([128, 512], BF16, tag=f"u1_{kc}", name=f"u1_{kc}")
                nc.sync.dma_start(u_sb[1][kc][:], ut_v[:, kc, 512:1024])

            xsq_sb = spool.tile([128, KC], FP32, tag="xsq_sb")

            for bc in range(BC):
                zps = []
                for nh in range(2):
                    zp = zpool.tile([128, 512], FP32, tag=f"zp{nh}", name=f"zp_{bc}_{nh}")
                    zps.append(zp)
                for kc in range(KC):
                    lhsT = x_sb[kc][:, bc * 128:(bc + 1) * 128]
                    for nh in range(2):
                        nc.tensor.matmul(
                            zps[nh][:], lhsT, u_sb[nh][kc][:],
                            start=(kc == 0), stop=(kc == KC - 1),
                        )
                for nh in range(2):
                    # e = z~^2 ; [128 batch, 512] -> per-subspace sums [128, 16]
                    e = epool.tile([128, 512], BF16, tag="e")
                    nc.scalar.square(e[:], zps[nh][:])
                    a = apool.tile([128, K], FP32, tag=f"a_{bc}_{nh}")
                    nc.vector.reduce_sum(
                        a[:], e.rearrange("p (k c) -> p k c", c=d),
                        axis=mybir.AxisListType.X,
                    )
                    m = mpool.tile([128, 1], FP32, tag=f"m_{bc}_{nh}")
                    nc.vector.reduce_max(m[:], a[:], axis=mybir.AxisListType.X)
                    nc.sync.dma_start(
                        mx.ap()[nh, bc * 128:(bc + 1) * 128].rearrange("(p o) -> p o", o=1),
                        m[:],
                    )
                # xsq chunks interleaved so ScalarE stays fed between e-squares
                for kc in range(2 * bc, 2 * bc + 2):
                    sq = sqpool.tile([128, NB], FP32, tag="sq")
                    nc.scalar.activation(
                        sq[:], x_sb[kc][:], mybir.ActivationFunctionType.Square,
                        accum_out=xsq_sb[:, kc:kc + 1],
                    )
            nc.sync.dma_start(xsq.ap()[:, :], xsq_sb[:])

    nc.compile()
    return nc


def _prep(x, Us):
    xt_bf = np.ascontiguousarray(x.T.astype(ml_dtypes.bfloat16))       # (D, B)
    Us64 = Us.astype(np.float64)
    eye = np.eye(d)
    # fold chol(I - 0.5 U^T U) into U, all 64 subspaces at once
    G = np.einsum('skDa,skDb->skab', Us64, Us64)                        # (R,K,d,d)
    L = np.linalg.cholesky(eye[None, None] - 0.5 * G)
    Ut = np.einsum('skDa,skab->skDb', Us64, L)                          # (R,K,D,d)
    ut_all = Ut.transpose(2, 0, 1, 3).reshape(D, R * K * d)             # (D, 2048)
    ut_bf = np.ascontiguousarray(ut_all.astype(ml_dtypes.bfloat16))
    in_maps = []
    for c in range(NCORES):
        s2, b4 = c // 4, c % 4
        in_maps.append({
            "xt": np.ascontiguousarray(xt_bf[:, NB * b4: NB * (b4 + 1)]),
            "ut": np.ascontiguousarray(ut_bf[:, 1024 * s2: 1024 * (s2 + 1)]),
        })
    return in_maps


def kernel(x, Us, _trace=False):
    global LAST_RESULTS
    if "nc" not in _COMPILED:
        _COMPILED["nc"] = _build()
    nc = _COMPILED["nc"]
    in_maps = _prep(np.asarray(x), np.asarray(Us))
    res = run_bass_kernel_spmd(nc, in_maps, core_ids=list(range(NCORES)),
                               trace=_trace)
    LAST_RESULTS = res
    S = sum(res.results[c]["xsq"].sum(dtype=np.float64) for c in range(4))
    base = 0.5 * S / B
    obj = np.empty(R, np.float32)
    for r in range(R):
        s2, nh = r // 2, r % 2
        vals = np.concatenate([res.results[4 * s2 + b]["mx"][nh] for b in range(4)])
        obj[r] = np.float32(base - vals.astype(np.float64).mean())
    return obj


# revision 4
# speedup vs baseline: 1.1934x; 1.1934x over previous
"""Trainium2 Bass kernel for the KSubspaceBaseModel objective.

Reference computes, for B=2048 samples x (B, D=1024) and subspace bases
Us (R=4, K=16, D, d=32):
    z = x @ U; x_ = z @ U^T; loss = 0.5*||x - x_||^2  (per b, r, k)
    obj_r = mean_b min_k loss

Algebraic collapse used here: with G = U^T U,
    loss = 0.5||x||^2 - z^T (I - 0.5 G) z
Folding L = chol(I - 0.5G) into U (Ut = U @ L) host-side gives
    loss = 0.5||x||^2 - ||Ut^T x||^2
so the device only computes z~ = Ut^T x, squares it, sums each subspace's
32 latent columns, and takes max_k.  obj_r = 0.5*mean||x||^2 - mean_b max_k.

Sharding over 8 cores: 4 batch quarters (512 samples) x 2 subspace halves
(32 subspaces = 2 whole replicates), so the k-max is core-local.
Device layout: stationary = x^T chunks (contraction D on partitions),
moving = Ut.  z~ lands [batch(128) x latent] in PSUM, so per-subspace sums
and the k-max are free-dim reductions.

DMA strategy: 3 coalesced 1MB input DMAs (ut half 0 + ut half 1 on the
sync HWDGE ring, xt on the scalar ring) and 2 coalesced output DMAs —
per-dma_start fixed cost is ~1-2us and transfers serialize FIFO per ring.
"""

import numpy as np
import ml_dtypes

import concourse.bass as bass
import concourse.bacc as bacc
import concourse.mybir as mybir
import concourse.tile as tile
from concourse.bass_utils import run_bass_kernel_spmd

B, D, R, K, d = 2048, 1024, 4, 16, 32
NCORES = 8
NB = B // 4          # 512 samples per core
NS = 32              # subspaces per core (2 replicates)
KC = D // 128        # 8 contraction chunks
BC = NB // 128       # 4 batch chunks per core
BF16 = mybir.dt.bfloat16
FP32 = mybir.dt.float32

_COMPILED = {}
LAST_RESULTS = None


def _build():
    nc = bacc.Bacc("TRN2", target_bir_lowering=False, debug=False)
    xt = nc.dram_tensor("xt", [D, NB], BF16, kind="ExternalInput")
    ut = nc.dram_tensor("ut", [D, NS * d], BF16, kind="ExternalInput")
    mx = nc.dram_tensor("mx", [128, 2 * BC], FP32, kind="ExternalOutput")
    xsq = nc.dram_tensor("xsq", [128, KC], FP32, kind="ExternalOutput")

    xt_v = xt.ap().rearrange("(o p) n -> p o n", p=128)   # [128, KC, NB]
    ut_v = ut.ap().rearrange("(o p) n -> p o n", p=128)   # [128, KC, 1024]

    with tile.TileContext(nc) as tc:
        with (
            tc.tile_pool(name="xsb", bufs=1) as xpool,
            tc.tile_pool(name="usb", bufs=1) as upool,
            tc.tile_pool(name="esb", bufs=3) as epool,
            tc.tile_pool(name="asb", bufs=1) as apool,
            tc.tile_pool(name="sqsb", bufs=2) as sqpool,
            tc.tile_pool(name="single", bufs=1) as spool,
            tc.tile_pool(name="zp", bufs=2, space="PSUM") as zpool,
        ):
            # 3 coalesced 1MB input DMAs across both HWDGE rings.
            u_sb = [None, None]
            u_sb[0] = upool.tile([128, KC, 512], BF16, tag="u0", name="u0")
            nc.sync.dma_start(u_sb[0][:], ut_v[:, :, 0:512])
            x_sb = xpool.tile([128, KC, NB], BF16, tag="x", name="x")
            nc.scalar.dma_start(x_sb[:], xt_v[:, :, :])
            u_sb[1] = upool.tile([128, KC, 512], BF16, tag="u1", name="u1")
            nc.sync.dma_start(u_sb[1][:], ut_v[:, :, 512:1024])

            xsq_sb = spool.tile([128, KC], FP32, tag="xsq_sb")
            stage = spool.tile([128, 2 * BC], FP32, tag="stage")

            for bc in range(BC):
                zps = []
                for nh in range(2):
                    zp = zpool.tile([128, 512], FP32, tag=f"zp{nh}",
                                    name=f"zp_{bc}_{nh}")
                    zps.append(zp)
                for kc in range(KC):
                    lhsT = x_sb[:, kc, bc * 128:(bc + 1) * 128]
                    for nh in range(2):
                        nc.tensor.matmul(
                            zps[nh][:], lhsT, u_sb[nh][:, kc, :],
                            start=(kc == 0), stop=(kc == KC - 1),
                        )
                for nh in range(2):
                    # e = z~^2 ; [128 batch, 512] -> per-subspace sums [128, 16]
                    e = epool.tile([128, 512], BF16, tag="e")
                    nc.scalar.square(e[:], zps[nh][:])
                    a = apool.tile([128, K], FP32, tag=f"a_{bc}_{nh}",
                                   name=f"a_{bc}_{nh}")
                    nc.vector.reduce_sum(
                        a[:], e.rearrange("p (k c) -> p k c", c=d),
                        axis=mybir.AxisListType.X,
                    )
                    # mx[p, 2*bc+nh] = max_k for sample bc*128+p, replicate nh
                    nc.vector.reduce_max(stage[:, 2 * bc + nh: 2 * bc + nh + 1],
                                         a[:], axis=mybir.AxisListType.X)
                # xsq chunks interleaved so ScalarE stays fed between e-squares
                for kc in range(2 * bc, 2 * bc + 2):
                    sq = sqpool.tile([128, NB], FP32, tag="sq")
                    nc.scalar.activation(
                        sq[:], x_sb[:, kc, :], mybir.ActivationFunctionType.Square,
                        accum_out=xsq_sb[:, kc:kc + 1],
                    )
            nc.sync.dma_start(mx.ap()[:, :], stage[:])
            nc.sync.dma_start(xsq.ap()[:, :], xsq_sb[:])

    nc.compile()
    return nc


def _prep(x, Us):
    xt_bf = np.ascontiguousarray(x.T.astype(ml_dtypes.bfloat16))       # (D, B)
    Us64 = Us.astype(np.float64)
    eye = np.eye(d)
    # fold chol(I - 0.5 U^T U) into U, all 64 subspaces at once
    G = np.einsum('skDa,skDb->skab', Us64, Us64)                        # (R,K,d,d)
    L = np.linalg.cholesky(eye[None, None] - 0.5 * G)
    Ut = np.einsum('skDa,skab->skDb', Us64, L)                          # (R,K,D,d)
    ut_all = Ut.transpose(2, 0, 1, 3).reshape(D, R * K * d)             # (D, 2048)
    ut_bf = np.ascontiguousarray(ut_all.astype(ml_dtypes.bfloat16))
    in_maps = []
    for c in range(NCORES):
        s2, b4 = c // 4, c % 4
        in_maps.append({
            "xt": np.ascontiguousarray(xt_bf[:, NB * b4: NB * (b4 + 1)]),
            "ut": np.ascontiguousarray(ut_bf[:, 1024 * s2: 1024 * (s2 + 1)]),
        })
    return in_maps


def kernel(x, Us, _trace=False):
    global LAST_RESULTS
    if "nc" not in _COMPILED:
        _COMPILED["nc"] = _build()
    nc = _COMPILED["nc"]
    in_maps = _prep(np.asarray(x), np.asarray(Us))
    res = run_bass_kernel_spmd(nc, in_maps, core_ids=list(range(NCORES)),
                               trace=_trace)
    LAST_RESULTS = res
    S = sum(res.results[c]["xsq"].sum(dtype=np.float64) for c in range(4))
    base = 0.5 * S / B
    obj = np.empty(R, np.float32)
    for r in range(R):
        s2, nh = r // 2, r % 2
        # mx[p, 2*bc+nh] = max_k of sample bc*128+p for replicate nh
        vals = [res.results[4 * s2 + b]["mx"][:, nh::2] for b in range(4)]
        obj[r] = np.float32(base - np.mean(
            [v.astype(np.float64).mean() for v in vals]))
    return obj
